# revision 5
# baseline (speedup 1.0000x reference)
"""Trainium2 Bass kernel for nn_BertEmbedding_1623497638029.

Per batch row b and token t (T=256 tokens, P=512 subword positions,
H=768), with subword counts lens in {0,1,2}:

    cum  = cumsum(bert_lens[b])
    lo_t = cum[t] - lens[t]        # first subword of token t
    hi_t = cum[t] - 1              # last subword of token t
    out[b,t] = mean(enc[b, lo_t:hi_t+1]) if lens[t] else 0

Implementation: tokens are processed as 128 PAIRS per batch row. One
indirect-DMA gather per batch row fetches a 4-row window per pair,
rows w..w+3 with w = max(cum[2p+1]-4, 0) (always in bounds since
cum <= P). The window covers both tokens' subword rows; each token's
output is a coefficient-weighted sum of the 4 window rows:

    m1 = min(cum[2p+1]-4, 0)            # window clamp shift (rarely nonzero)
    pos_hi1 = 3+m1           pos_lo1 = 4-l1+m1
    pos_hi0 = 3-l1+m1        pos_lo0 = 4-l1-l0+m1
    out_even = sum_k u_k r_k,  u_k = b0*[pos_hi0==k] + a0*[pos_lo0==k]
    out_odd  = sum_k v_k r_k,  v_k = b1*[pos_hi1==k] + a1*[pos_lo1==k]
    a = 0.5*(len>1),  b = (len>0) - a   (per token)

At most two coefficients per token are nonzero (0.5/0.5 for len==2,
one 1.0 for len==1, none for padding), so every output element is a
single-rounded sum of at most two scaled rows — bit-identical to the
reference segment-mean.

The pair-layout quantities cum[2p+1], lens[2p], lens[2p+1] are computed
on-chip with selection-matrix matmuls (PE) from a transposed lens tile;
the selection masks are affine_select constants.

This shape (8 gathers of 1.5 MB with 12 KB descriptors + 8 contiguous
768 KB stores per core) measured fastest on hardware versus per-token
2-row gathers (16 DMAs) and per-row gathers with OOB-skipped
accumulation (32 DMAs): fewer indirect-DMA instructions beat smaller
total descriptor counts, and skipped-descriptor byte savings did not
offset the extra per-instruction overhead.

Sharding: pure data parallel — 8 batch rows per NeuronCore, 8 cores,
no cross-core communication.
"""

import numpy as np

import concourse.bacc as bacc
import concourse.bass as bass
import concourse.mybir as mybir
import concourse.tile as tile
from concourse.bass_utils import run_bass_kernel_spmd
from concourse.masks import make_identity

NCORES = 8
BZ, P, T, H = 64, 512, 256, 768
BL = BZ // NCORES  # batch rows per core
NCH = T // 128  # 128-token chunks per batch row

F32 = mybir.dt.float32
F16 = mybir.dt.float16
I32 = mybir.dt.int32
ALU = mybir.AluOpType
AF = mybir.ActivationFunctionType


def _sel_mask(nc, t, base):
    """t[k,p] = 1 iff base + k - 2p == 0."""
    nc.gpsimd.memset(t, 0.0)
    nc.gpsimd.affine_select(
        out=t, in_=t, compare_op=ALU.not_equal, fill=1.0,
        base=base, pattern=[[-2, 128]], channel_multiplier=1,
    )


def _le_mask(nc, t, base):
    """t[k,p] = 1 iff base + k - 2p <= 0."""
    nc.gpsimd.memset(t, 0.0)
    nc.gpsimd.affine_select(
        out=t, in_=t, compare_op=ALU.is_gt, fill=1.0,
        base=base, pattern=[[-2, 128]], channel_multiplier=1,
    )


def _build_nc(acc_bufs=6, res_bufs=4, tmp_bufs=2, repeat=0, asserts=True,
              pool_add=False):
    nc = bacc.Bacc(
        "TRN2", target_bir_lowering=False, debug=False,
        num_devices=NCORES, enable_asserts=asserts,
    )
    enc = nc.dram_tensor("enc", [BL * P, H], F16, kind="ExternalInput").ap()
    lens = nc.dram_tensor("lens", [BL, T], I32, kind="ExternalInput").ap()
    out = nc.dram_tensor("out", [BL, T, H], F16, kind="ExternalOutput").ap()

    with tile.TileContext(nc) as tc:
        with (
            tc.tile_pool(name="const", bufs=1) as cpool,
            tc.tile_pool(name="idx", bufs=1) as ipool,
            tc.tile_pool(name="psum", bufs=2, space="PSUM") as ppool,
            tc.tile_pool(name="acc", bufs=acc_bufs) as apool,
            tc.tile_pool(name="tmp", bufs=tmp_bufs) as tpool,
            tc.tile_pool(name="res", bufs=res_bufs) as rpool,
        ):
            # ---- constants ----
            ident = cpool.tile([128, 128], F32)
            make_identity(nc, ident[:])
            # selection/prefix masks: columns are pairs p, rows are chunk-local k
            mcum0 = cpool.tile([128, 128], F32)  # k <= 2p+1 (chunk 0)
            _le_mask(nc, mcum0[:], base=-1)
            mcum1 = cpool.tile([128, 128], F32)  # 128+k <= 2p+1
            _le_mask(nc, mcum1[:], base=127)
            se0 = cpool.tile([128, 128], F32)  # k == 2p (chunk 0)
            _sel_mask(nc, se0[:], base=0)
            se1 = cpool.tile([128, 128], F32)  # 128+k == 2p
            _sel_mask(nc, se1[:], base=128)
            so0 = cpool.tile([128, 128], F32)  # k == 2p+1 (chunk 0)
            _sel_mask(nc, so0[:], base=-1)
            so1 = cpool.tile([128, 128], F32)  # 128+k == 2p+1
            _sel_mask(nc, so1[:], base=127)
            boff_i = cpool.tile([128, BL], I32)  # per-column batch row offset
            nc.gpsimd.iota(boff_i[:], pattern=[[P, BL]], base=0, channel_multiplier=0)
            boff = cpool.tile([128, BL], F32)
            nc.vector.tensor_copy(boff[:], boff_i[:])
            badj = cpool.tile([1, BL], F32)  # 512*n - 4 per column
            nc.vector.tensor_scalar_add(badj[:], boff[0:1, :], -4.0)

            # ---- load lens, cast, transpose chunks to [token, batch] ----
            lens_i = ipool.tile([BL, T], I32)
            nc.sync.dma_start(out=lens_i[:], in_=lens[:, :])
            lens_f = ipool.tile([BL, T], F32)
            nc.vector.tensor_copy(lens_f[:], lens_i[:])
            lensT = []
            for c in range(NCH):
                ps_t = ppool.tile([128, BL], F32, tag="tr")
                nc.tensor.transpose(
                    out=ps_t[:], in_=lens_f[:, c * 128 : (c + 1) * 128],
                    identity=ident[0:BL, 0:BL],
                )
                lt = ipool.tile([128, BL], F32, tag=f"lensT{c}")
                nc.vector.tensor_copy(lt[:], ps_t[:])
                lensT.append(lt)

            # ---- pair-layout quantities via selection matmuls ----
            def _accum(masks, name, extra=None):
                pt = ppool.tile([128, BL], F32, tag=name)
                nc.tensor.matmul(out=pt[:], lhsT=masks[0][:], rhs=lensT[0][:],
                                 start=True, stop=False)
                nc.tensor.matmul(out=pt[:], lhsT=masks[1][:], rhs=lensT[1][:],
                                 start=False, stop=(extra is None))
                if extra is not None:
                    # rank-1 accumulate: mcum0 row 0 is all ones
                    nc.tensor.matmul(out=pt[:], lhsT=mcum0[0:1, :],
                                     rhs=extra[:], start=False, stop=True)
                return pt

            c1v = _accum((mcum0, mcum1), "c1v", extra=badj)  # cum[2p+1]-4+boff
            l0v = _accum((se0, se1), "l0v")      # lens[2p]
            l1v = _accum((so0, so1), "l1v")      # lens[2p+1]

            # ---- window index and coefficients (all [128, BL] f32) ----
            # c1v holds cum-4+boff; wg = max(cum-4,0)+boff = max(c1v, boff)
            wg = ipool.tile([128, BL], I32)
            nc.vector.tensor_tensor(out=wg[:], in0=c1v[:], in1=boff[:],
                                    op=ALU.max)
            m1 = ipool.tile([128, BL], F32)  # min(cum-4, 0) = min(c1v-boff, 0)
            nc.vector.tensor_sub(m1[:], c1v[:], boff[:])
            nc.vector.tensor_scalar_min(m1[:], m1[:], 0.0)

            x = ipool.tile([128, BL], F32)  # m1 - l1
            nc.vector.tensor_sub(x[:], m1[:], l1v[:])
            pos_hi1 = ipool.tile([128, BL], F32)
            nc.vector.tensor_scalar_add(pos_hi1[:], m1[:], 3.0)
            pos_hi0 = ipool.tile([128, BL], F32)
            nc.vector.tensor_scalar_add(pos_hi0[:], x[:], 3.0)
            pos_lo1 = ipool.tile([128, BL], F32)
            nc.vector.tensor_scalar_add(pos_lo1[:], x[:], 4.0)
            pos_lo0 = ipool.tile([128, BL], F32)
            nc.vector.tensor_sub(pos_lo0[:], pos_lo1[:], l0v[:])

            def ab(lv, tag):
                a = ipool.tile([128, BL], F32, tag=f"a{tag}")
                nc.vector.tensor_scalar(out=a[:], in0=lv[:], scalar1=1.0,
                                        scalar2=0.5, op0=ALU.is_gt, op1=ALU.mult)
                g = ipool.tile([128, BL], F32, tag=f"g{tag}")
                nc.vector.tensor_scalar(out=g[:], in0=lv[:], scalar1=0.0,
                                        scalar2=None, op0=ALU.is_gt)
                b = ipool.tile([128, BL], F32, tag=f"b{tag}")
                nc.vector.tensor_sub(b[:], g[:], a[:])
                return a, b

            a0, b0 = ab(l0v, "0")
            a1, b1 = ab(l1v, "1")

            def coef(k, poshi, poslo, av, bv, tag):
                ih = ipool.tile([128, BL], F32, tag=f"ih{tag}{k}")
                nc.vector.tensor_scalar(out=ih[:], in0=poshi[:],
                                        scalar1=float(k), scalar2=None,
                                        op0=ALU.is_equal)
                il = ipool.tile([128, BL], F32, tag=f"il{tag}{k}")
                nc.vector.tensor_scalar(out=il[:], in0=poslo[:],
                                        scalar1=float(k), scalar2=None,
                                        op0=ALU.is_equal)
                u = ipool.tile([128, BL], F32, tag=f"u{tag}{k}")
                nc.vector.tensor_mul(u[:], ih[:], bv[:])
                t2 = ipool.tile([128, BL], F32, tag=f"t2{tag}{k}")
                nc.vector.tensor_mul(t2[:], il[:], av[:])
                nc.vector.tensor_add(u[:], u[:], t2[:])
                return u

            u = [coef(k, pos_hi0, pos_lo0, a0, b0, "u") for k in range(4)]
            v = [coef(k, pos_hi1, pos_lo1, a1, b1, "v") for k in range(4)]

            # ---- main loop: one 4-row-window gather + combine per batch ----
            def main_body(_iv=None):
                for b in range(BL):
                    acc = apool.tile([128, 4 * H], F16, tag="acc")
                    nc.gpsimd.indirect_dma_start(
                        out=acc[:], out_offset=None, in_=enc[:, :],
                        in_offset=bass.IndirectOffsetOnAxis(
                            ap=wg[:, b : b + 1], axis=0),
                    )
                    res = rpool.tile([128, 2 * H], F16, tag="res")
                    r = [acc[:, k * H : (k + 1) * H] for k in range(4)]
                    for parity, cf in ((0, u), (1, v)):
                        sl = res[:, parity * H : (parity + 1) * H]
                        # sl = cf0*r0 + (cf1*r1 + (cf2*r2 + cf3*r3))
                        t3 = tpool.tile([128, H], F16, tag=f"t3{parity}")
                        nc.scalar.activation(out=t3[:], in_=r[3], func=AF.Copy,
                                             scale=cf[3][:, b : b + 1])
                        t2 = tpool.tile([128, H], F16, tag=f"t2{parity}")
                        nc.vector.scalar_tensor_tensor(
                            out=t2[:], in0=r[2], scalar=cf[2][:, b : b + 1],
                            in1=t3[:], op0=ALU.mult, op1=ALU.add)
                        t1 = tpool.tile([128, H], F16, tag=f"t1{parity}")
                        if pool_add:
                            nc.scalar.activation(out=t1[:], in_=r[1],
                                                 func=AF.Copy,
                                                 scale=cf[1][:, b : b + 1])
                            nc.gpsimd.tensor_add(t1[:], t1[:], t2[:])
                        else:
                            nc.vector.scalar_tensor_tensor(
                                out=t1[:], in0=r[1],
                                scalar=cf[1][:, b : b + 1],
                                in1=t2[:], op0=ALU.mult, op1=ALU.add)
                        nc.vector.scalar_tensor_tensor(
                            out=sl, in0=r[0], scalar=cf[0][:, b : b + 1],
                            in1=t1[:], op0=ALU.mult, op1=ALU.add)
                    dest = out[b : b + 1, :, :].rearrange(
                        "o (tp q) h -> (o tp) q h", q=2
                    )
                    nc.sync.dma_start(out=dest, in_=res[:].rearrange(
                        "p (q h) -> p q h", h=H))

            if repeat:
                # timing mode: run the steady-state body `repeat` times
                tc.For_i_unrolled(0, repeat, 1, main_body, max_unroll=2)
            else:
                main_body()

    nc.compile()
    return nc


_NC = None


def _get_nc():
    global _NC
    if _NC is None:
        _NC = _build_nc()
    return _NC


def kernel(enc_out, bert_mask, bert_lens):
    del bert_mask  # implied by bert_lens (mask = arange(P) < cumsum(lens)[-1])
    enc_np = np.ascontiguousarray(np.asarray(enc_out, dtype=np.float32))
    lens_np = np.ascontiguousarray(np.asarray(bert_lens, dtype=np.int32))
    assert enc_np.shape == (BZ, P, H) and lens_np.shape == (BZ, T)
    enc_np = enc_np.astype(np.float16)

    nc = _get_nc()
    in_maps = [
        {
            "enc": enc_np[i * BL : (i + 1) * BL].reshape(BL * P, H),
            "lens": lens_np[i * BL : (i + 1) * BL],
        }
        for i in range(NCORES)
    ]
    results = run_bass_kernel_spmd(nc, in_maps, core_ids=list(range(NCORES))).results
    out = np.concatenate([np.asarray(r["out"]) for r in results], axis=0)
    return out.reshape(BZ, T, H).astype(np.float32)

